# revision 14
# baseline (speedup 1.0000x reference)
"""CQT (constant-Q transform) + amplitude_to_db kernel for Trainium2.

Full-input contract: kernel(x) takes x [32, 64000] f32 and returns
[32, 84, 126] f32, matching:

    frames = pad(x, n_fft//2)[:, t*HOP + n]          # [B, 126, 16384]
    cr/ci  = frames @ Kr.T / Ki.T                    # [B, 84, 126]
    mag    = sqrt(cr^2 + ci^2)
    out    = amplitude_to_db(mag, ref=max per item, amin=1e-5, top_db=80)

Sharding: pure data parallelism — 4 batch items per NeuronCore on 8 cores.

Per-core compute layout:
  * The frame/filter contraction is one big matmul with K = n_fft = 16384,
    contracted in 128-row chunks. The frame matrix is never materialized:
    padded x stored column-major in SBUF ([128, 628] with x_cm[p,f] =
    xp[f*128+p]) makes chunk c of frames^T a strided AP view
    x_cm[:, c : c+501 : 4] (HOP=512 = 4*128).
  * CQT kernels are ~85% zeros (row k nonzero only in a centered window of
    length L_k, L_k halving per octave). Bins are split into two groups so
    only K-chunks intersecting each group's support are computed:
      group A: bins 0..63  (re+im packed on M: M=128), ~90 chunks
      group B: bins 64..83 (M=64, 32-aligned re/im halves), ~4 chunks
    All 4 items share each matmul via N = 4*126 = 504 <= 512.
  * dB epilogue: m2 = cr^2+ci^2, clamp at amin^2, ACT Ln, per-item max via
    free-dim reduce + GpSimd partition all-reduce, then
    out = max((ln(m2c) - ln(ref2c)) * 10/ln(10), -80).
"""

import os
import numpy as np
import ml_dtypes
from contextlib import ExitStack

import concourse.bass as bass
import concourse.mybir as mybir
import concourse.tile as tile
from concourse import bacc
from concourse import bass_isa
from concourse.bass_utils import run_bass_kernel_spmd

# matmul input dtype: fp16 keeps the PE at full rate (1 col/cycle) with a
# 10-bit mantissa; bf16 is the fallback; fp32r is the full-precision
# replicated-fp32 PE mode (full rate at N>=256 per the cost model).
MM_DTYPE = os.environ.get("CQT_MM_DTYPE", "fp16")
_DTYPES = {
    "bf16": (mybir.dt.bfloat16, ml_dtypes.bfloat16),
    "fp16": (mybir.dt.float16, np.float16),
    "fp32r": (mybir.dt.float32r, np.float32),
    "fp32": (mybir.dt.float32, np.float32),
}
MM_DT, MM_NP = _DTYPES[MM_DTYPE]

# fp16's normal range bottoms out at 6.1e-5, but low-bin CQT weights peak at
# ~1.7e-6 — pre-scale all weights by 2^14 to clear the subnormal range. The
# scale cancels exactly in the ref-normalized dB output; only the amin clamp
# constant has to be scaled to match.
W_SCALE = 2.0 ** 14 if MM_DTYPE == "fp16" else 1.0

# ---- problem constants (hardcoded; must match the reference) ----
SR = 22050
HOP = 512
N_BINS = 84
BPO = 12
FMIN = 32.70319566257483
AMIN = 1e-5
TOP_DB = 80.0
B = 32
N_SAMP = 64000
N_CORES = 8
NI = B // N_CORES            # items per core = 4
T = 1 + N_SAMP // HOP        # 126 frames
DB_SCALE = 10.0 / np.log(10.0)  # 20*log10(mag) == DB_SCALE * ln(mag^2)

P = 128
SPLIT_BIN = 64               # group A: bins [0,64), group B: bins [64,84)
NB_BINS = N_BINS - SPLIT_BIN  # 20
MB = 64                      # group B stationary width (re at 0:20, im at 32:52)


def _build_cqt_kernels():
    """Same construction as the reference (nnAudio-style direct CQT bank)."""
    Q = 1.0 / (2.0 ** (1.0 / BPO) - 1.0)
    freqs = FMIN * 2.0 ** (np.arange(N_BINS) / BPO)
    lengths = np.ceil(Q * SR / freqs).astype(int)
    n_fft = int(2 ** np.ceil(np.log2(lengths.max())))
    K = np.zeros((N_BINS, n_fft), dtype=np.complex128)
    for k in range(N_BINS):
        L = int(lengths[k])
        t = np.arange(L) - (L - 1) / 2.0
        kern = np.hanning(L) * np.exp(2j * np.pi * freqs[k] * t / SR)
        kern /= np.abs(kern).sum()
        kern /= np.sqrt(L)
        s = (n_fft - L) // 2
        K[k, s:s + L] = kern
    return K.real.astype(np.float32), K.imag.astype(np.float32), n_fft


def _chunk_range(Kr, Ki, bins):
    """Contiguous range of 128-row K-chunks with any nonzero for these bins."""
    nz = (np.abs(Kr[bins]).max(axis=0) + np.abs(Ki[bins]).max(axis=0)) > 0
    idx = np.nonzero(nz)[0]
    return int(idx[0]) // P, int(idx[-1]) // P + 1


Kr, Ki, N_FFT = _build_cqt_kernels()
PAD = N_FFT // 2
FW = (N_SAMP + 2 * PAD) // P      # 628 free-dim width of column-major xp
assert (N_SAMP + 2 * PAD) % P == 0 and HOP == 4 * P

_A0, _A1 = _chunk_range(Kr, Ki, range(0, SPLIT_BIN))
_B0, _B1 = _chunk_range(Kr, Ki, range(SPLIT_BIN, N_BINS))
# Chunks are processed grouped by phase r = c % 4 (ascending), matching the
# order the four phase-DMAs of x land in SBUF, so the first matmuls only
# wait for the first phase slice instead of the whole signal. The four
# B-group chunks are interleaved into the same phase order so they never
# stall the PE FIFO waiting for a late phase.
CHUNKS_A = sorted(range(_A0, _A1), key=lambda c: (c % 4, c))  # 90 chunks
CHUNKS_B = sorted(range(_B0, _B1), key=lambda c: (c % 4, c))  # 4 chunks
NA = len(CHUNKS_A)
NB = len(CHUNKS_B)
SCHEDULE = sorted([("A", c) for c in CHUNKS_A] + [("B", c) for c in CHUNKS_B],
                  key=lambda gc: (gc[1] % 4, gc[1]))
# Weight DMA pieces: small leading pieces so the first matmuls only wait for
# a few chunks of weights; later pieces are big to keep issue-op count low.
SLAB_SIZES = [4, 4, 12, 12, 20, 19, 19]
assert sum(SLAB_SIZES) == NA
SLAB_OFF = [0]
for _sz in SLAB_SIZES:
    SLAB_OFF.append(SLAB_OFF[-1] + _sz)
N_SLABS = len(SLAB_SIZES)


def _slab_of(j):
    for s in range(N_SLABS):
        if j < SLAB_OFF[s + 1]:
            return s, j - SLAB_OFF[s]
    raise IndexError(j)


def _pack_weights():
    KrT = Kr.T  # [N_FFT, 84]
    KiT = Ki.T
    wa = np.zeros((P, NA * P), np.float32)
    for j, c in enumerate(CHUNKS_A):
        wa[:, j * P: j * P + SPLIT_BIN] = KrT[c * P:(c + 1) * P, :SPLIT_BIN]
        wa[:, j * P + SPLIT_BIN:(j + 1) * P] = KiT[c * P:(c + 1) * P, :SPLIT_BIN]
    wb = np.zeros((P, NB * MB), np.float32)
    for j, c in enumerate(CHUNKS_B):
        wb[:, j * MB: j * MB + NB_BINS] = KrT[c * P:(c + 1) * P, SPLIT_BIN:]
        wb[:, j * MB + 32: j * MB + 32 + NB_BINS] = KiT[c * P:(c + 1) * P, SPLIT_BIN:]
    return (wa * W_SCALE).astype(MM_NP), (wb * W_SCALE).astype(MM_NP)


WA, WB = _pack_weights()


def build_program():
    """One-core SPMD program (identical on all cores; data differs)."""
    nc = bacc.Bacc("TRN2", target_bir_lowering=False, debug=False,
                   enable_asserts=True)
    bf16 = MM_DT
    f32 = mybir.dt.float32

    x_in = nc.dram_tensor("x_in", [4, P, NI * FW // 4], bf16,
                          kind="ExternalInput").ap()
    wa_in = nc.dram_tensor("wa_in", [P, NA * P], bf16, kind="ExternalInput").ap()
    wb_in = nc.dram_tensor("wb_in", [P, NB * MB], bf16, kind="ExternalInput").ap()
    # [bins, items, T] so the result leaves SBUF as ONE contiguous DMA;
    # the host transposes to [items, bins, T] afterwards (free in numpy).
    out = nc.dram_tensor("out", [N_BINS, NI, T], f32, kind="ExternalOutput").ap()

    with tile.TileContext(nc) as tc, ExitStack() as ctx:
        sb = ctx.enter_context(tc.tile_pool(name="sb", bufs=1))
        ps = ctx.enter_context(tc.tile_pool(name="ps", bufs=1, space="PSUM"))

        # input signal, phase-major: xt[p, r*628 + i*157 + q] = xp_i[(4q+r)*128+p].
        # DMAs are issued first (best scheduler priority), spread over the
        # sync/scalar/gpsimd rings roughly in consumption order.
        QW = FW // 4  # 157
        xt = sb.tile([P, NI * FW], bf16, name="xt")
        wbt = sb.tile([P, NB * MB], bf16, name="wbt")
        slabs = []
        for s in range(N_SLABS):
            slabs.append(sb.tile([P, SLAB_SIZES[s] * P], bf16, name=f"wa{s}"))

        def x_phase_dma(eng, r):
            eng.dma_start(xt[:, r * NI * QW:(r + 1) * NI * QW], x_in[r])

        def slab_dma(eng, s):
            c0, c1 = SLAB_OFF[s], SLAB_OFF[s + 1]
            eng.dma_start(slabs[s][:], wa_in[:, c0 * P:c1 * P])

        x_phase_dma(nc.sync, 0)
        nc.scalar.dma_start(wbt[:], wb_in)
        x_phase_dma(nc.gpsimd, 3)
        x_phase_dma(nc.sync, 1)
        x_phase_dma(nc.scalar, 2)
        slab_dma(nc.sync, 0)
        slab_dma(nc.scalar, 1)
        slab_dma(nc.sync, 2)
        slab_dma(nc.scalar, 3)
        slab_dma(nc.sync, 4)
        slab_dma(nc.scalar, 5)
        slab_dma(nc.gpsimd, 6)
        xv = xt[:].rearrange("p (r i q) -> p r i q", r=4, i=NI)

        # PE warmup: dummy matmuls during the DMA head so HAM unthrottles
        # (1.2 -> 2.4 GHz) close to the first real matmul.
        junk = sb.tile([P, 512], bf16, name="junk")
        nc.gpsimd.memset(junk[:], 0.0)
        psW = ps.tile([P, 504], f32, name="psW")
        for _ in range(3):
            nc.tensor.matmul(psW[:], lhsT=junk[:, :P], rhs=junk[:, :504],
                             start=True, stop=True)

        # Load the Ln table set early: a [1,1] dummy Ln is the first ACT op,
        # so its ACT_TABLE_LOAD hoists into the DMA head; Copy (table filler
        # present in every set) then reuses the resident set.
        lnwarm = sb.tile([1, 1], f32, name="lnwarm")
        nc.scalar.activation(lnwarm[:], nc.const_aps.tensor(1.0, (1, 1)),
                             mybir.ActivationFunctionType.Ln)

        def rhs_for(c):
            # x is stored phase-deinterleaved (see pack_x): chunk c's moving
            # operand is contiguous in t, so the PE streams at full rate
            # (a stride-4 AP measured 3 cycles/column instead of 1).
            r, q0 = c % 4, c // 4
            return xv[:, r, :, q0: q0 + T]  # [128, NI, T]

        # 94 accumulating matmuls in one phase-ordered stream; A chunks index
        # their slab by position in CHUNKS_A (slabs are packed in that order),
        # B chunks index wbt by position in CHUNKS_B.
        psA = ps.tile([P, NI, T], f32, name="psA")
        psB = ps.tile([MB, NI, T], f32, name="psB")
        na_seen = nb_seen = 0
        for grp, c in SCHEDULE:
            if grp == "A":
                j = CHUNKS_A.index(c)
                s, o = _slab_of(j)
                nc.tensor.matmul(psA[:], lhsT=slabs[s][:, o * P:(o + 1) * P],
                                 rhs=rhs_for(c), start=(na_seen == 0),
                                 stop=(na_seen == NA - 1))
                na_seen += 1
            else:
                j = CHUNKS_B.index(c)
                nc.tensor.matmul(psB[:], lhsT=wbt[:, j * MB:(j + 1) * MB],
                                 rhs=rhs_for(c), start=(nb_seen == 0),
                                 stop=(nb_seen == NB - 1))
                nb_seen += 1

        psAf = psA[:].rearrange("p i f -> p (i f)")
        psBf = psB[:].rearrange("p i f -> p (i f)")
        NT = NI * T

        # m2 = re^2 + im^2 for all 84 bins. ACT copies PSUM->SBUF (walrus
        # allows only one PSUM operand per DVE op); DVE squares and adds.
        # Copy is table-set-free filler, so the kernel's single table load
        # (Ln's set) hoists off the critical tail. (DVE tensor_tensor needs
        # equal base partitions for its SBUF inputs -> cre/cim mirror m2.)
        Copy = mybir.ActivationFunctionType.Copy
        cre = sb.tile([N_BINS, NT], f32, name="cre")
        cim = sb.tile([N_BINS, NT], f32, name="cim")
        m2 = sb.tile([N_BINS, NT], f32, name="m2")
        tmp = sb.tile([N_BINS, NT], f32, name="tmp")
        nc.scalar.activation(cre[SPLIT_BIN:], psBf[:NB_BINS], Copy)
        nc.scalar.activation(cim[SPLIT_BIN:], psBf[32:32 + NB_BINS], Copy)
        nc.vector.tensor_mul(m2[SPLIT_BIN:], cre[SPLIT_BIN:], cre[SPLIT_BIN:])
        nc.vector.tensor_mul(tmp[SPLIT_BIN:], cim[SPLIT_BIN:], cim[SPLIT_BIN:])
        nc.vector.tensor_add(m2[SPLIT_BIN:], m2[SPLIT_BIN:], tmp[SPLIT_BIN:])
        nc.vector.tensor_copy(cim[:SPLIT_BIN], psAf[SPLIT_BIN:])
        nc.scalar.activation(cre[:SPLIT_BIN], psAf[:SPLIT_BIN], Copy)
        nc.vector.tensor_mul(m2[:SPLIT_BIN], psAf[:SPLIT_BIN], cre[:SPLIT_BIN])
        nc.vector.tensor_mul(tmp[:SPLIT_BIN], psAf[SPLIT_BIN:], cim[:SPLIT_BIN])
        nc.vector.tensor_add(m2[:SPLIT_BIN], m2[:SPLIT_BIN], tmp[:SPLIT_BIN])

        # clamp at amin^2 (amplitude_to_db's amin on mag == amin^2 on mag^2)
        nc.vector.tensor_scalar_max(m2[:], m2[:], float(AMIN * W_SCALE) ** 2)

        # ln(m2c) for every element
        lnm = sb.tile([N_BINS, NT], f32, name="lnm")
        nc.scalar.activation(lnm[:], m2[:], mybir.ActivationFunctionType.Ln)

        # per-item ref^2: free-dim max within each item block, then
        # partition all-reduce so every partition holds the item max
        r1 = sb.tile([N_BINS, NI], f32, name="r1")
        nc.vector.tensor_reduce(r1[:], m2[:].rearrange("p (i f) -> p i f", i=NI),
                                axis=mybir.AxisListType.X, op=mybir.AluOpType.max)
        rall = sb.tile([N_BINS, NI], f32, name="rall")
        nc.gpsimd.partition_all_reduce(rall[:], r1[:], channels=N_BINS,
                                       reduce_op=bass_isa.ReduceOp.max)
        lnr = sb.tile([N_BINS, NI], f32, name="lnr")
        nc.scalar.activation(lnr[:], rall[:], mybir.ActivationFunctionType.Ln)

        # db = (ln(m2c) - ln(ref2c)) * 10/ln(10), clamped at -top_db
        db = sb.tile([N_BINS, NT], f32, name="db")
        for i in range(NI):
            nc.vector.tensor_scalar(db[:, i * T:(i + 1) * T],
                                    lnm[:, i * T:(i + 1) * T],
                                    lnr[:, i:i + 1], float(DB_SCALE),
                                    mybir.AluOpType.subtract,
                                    mybir.AluOpType.mult)

        outf = out.rearrange("k i t -> k (i t)")
        nc.scalar.dma_start(outf[:, :2 * T], db[:, :2 * T])
        nc.sync.dma_start(outf[:, 2 * T:], db[:, 2 * T:])

    nc.compile()
    return nc


def pack_x(x):
    """x [B, 64000] f32 -> per-core MM_DTYPE column-major packs [P, NI*FW]."""
    xp = np.pad(np.asarray(x, dtype=np.float32), ((0, 0), (PAD, PAD)))
    # phase-deinterleaved column-major: x_cm[b, p, r, q] = xp[b, (4q+r)*128+p]
    # so chunk c (= 4*q0 + r) streams contiguously in t (HOP = 4*128).
    x_cm = xp.reshape(B, FW // 4, 4, P).transpose(0, 3, 2, 1)  # [B,128,4,157]
    x_cm = x_cm.astype(MM_NP)
    packs = []
    for core in range(N_CORES):
        blk = x_cm[core * NI:(core + 1) * NI]           # [NI, 128, 4, 157]
        packs.append(np.ascontiguousarray(
            blk.transpose(2, 1, 0, 3).reshape(4, P, NI * (FW // 4))))
    return packs


_PROGRAM = None


def _get_program():
    global _PROGRAM
    if _PROGRAM is None:
        _PROGRAM = build_program()
    return _PROGRAM


def run(x, **spmd_kwargs):
    """Run on 8 NeuronCores; returns (output [32, 84, 126] f32, BassKernelResults)."""
    nc = _get_program()
    packs = pack_x(x)
    in_maps = [{"x_in": packs[i], "wa_in": WA, "wb_in": WB}
               for i in range(N_CORES)]
    res = run_bass_kernel_spmd(nc, in_maps, core_ids=list(range(N_CORES)),
                               **spmd_kwargs)
    out = np.concatenate([res.results[i]["out"].transpose(1, 0, 2)
                          for i in range(N_CORES)], axis=0)
    return np.ascontiguousarray(out.astype(np.float32)), res


def kernel(x):
    return run(x)[0]


# revision 15
# speedup vs baseline: 1.1432x; 1.1432x over previous
"""CQT (constant-Q transform) + amplitude_to_db kernel for Trainium2.

Full-input contract: kernel(x) takes x [32, 64000] f32 and returns
[32, 84, 126] f32, matching:

    frames = pad(x, n_fft//2)[:, t*HOP + n]          # [B, 126, 16384]
    cr/ci  = frames @ Kr.T / Ki.T                    # [B, 84, 126]
    mag    = sqrt(cr^2 + ci^2)
    out    = amplitude_to_db(mag, ref=max per item, amin=1e-5, top_db=80)

Sharding: pure data parallelism — 4 batch items per NeuronCore on 8 cores.

Per-core compute layout:
  * The frame/filter contraction is one big matmul with K = n_fft = 16384,
    contracted in 128-row chunks. The frame matrix is never materialized:
    padded x stored column-major in SBUF ([128, 628] with x_cm[p,f] =
    xp[f*128+p]) makes chunk c of frames^T a strided AP view
    x_cm[:, c : c+501 : 4] (HOP=512 = 4*128).
  * CQT kernels are ~85% zeros (row k nonzero only in a centered window of
    length L_k, L_k halving per octave). Bins are split into two groups so
    only K-chunks intersecting each group's support are computed:
      group A: bins 0..63  (re+im packed on M: M=128), ~90 chunks
      group B: bins 64..83 (M=64, 32-aligned re/im halves), ~4 chunks
    All 4 items share each matmul via N = 4*126 = 504 <= 512.
  * dB epilogue: m2 = cr^2+ci^2, clamp at amin^2, ACT Ln, per-item max via
    free-dim reduce + GpSimd partition all-reduce, then
    out = max((ln(m2c) - ln(ref2c)) * 10/ln(10), -80).
"""

import os
import numpy as np
import ml_dtypes
from contextlib import ExitStack

import concourse.bass as bass
import concourse.mybir as mybir
import concourse.tile as tile
from concourse import bacc
from concourse import bass_isa
from concourse.bass_utils import run_bass_kernel_spmd

# matmul input dtype: fp16 keeps the PE at full rate (1 col/cycle) with a
# 10-bit mantissa; bf16 is the fallback; fp32r is the full-precision
# replicated-fp32 PE mode (full rate at N>=256 per the cost model).
MM_DTYPE = os.environ.get("CQT_MM_DTYPE", "fp16")
_DTYPES = {
    "bf16": (mybir.dt.bfloat16, ml_dtypes.bfloat16),
    "fp16": (mybir.dt.float16, np.float16),
    "fp32r": (mybir.dt.float32r, np.float32),
    "fp32": (mybir.dt.float32, np.float32),
}
MM_DT, MM_NP = _DTYPES[MM_DTYPE]

# fp16's normal range bottoms out at 6.1e-5, but low-bin CQT weights peak at
# ~1.7e-6 — pre-scale all weights by 2^14 to clear the subnormal range. The
# scale cancels exactly in the ref-normalized dB output; only the amin clamp
# constant has to be scaled to match.
W_SCALE = 2.0 ** 14 if MM_DTYPE == "fp16" else 1.0

# ---- problem constants (hardcoded; must match the reference) ----
SR = 22050
HOP = 512
N_BINS = 84
BPO = 12
FMIN = 32.70319566257483
AMIN = 1e-5
TOP_DB = 80.0
B = 32
N_SAMP = 64000
N_CORES = 8
NI = B // N_CORES            # items per core = 4
T = 1 + N_SAMP // HOP        # 126 frames
DB_SCALE = 10.0 / np.log(10.0)  # 20*log10(mag) == DB_SCALE * ln(mag^2)

P = 128
SPLIT_BIN = 64               # group A: bins [0,64), group B: bins [64,84)
NB_BINS = N_BINS - SPLIT_BIN  # 20
MB = 64                      # group B stationary width (re at 0:20, im at 32:52)


def _build_cqt_kernels():
    """Same construction as the reference (nnAudio-style direct CQT bank)."""
    Q = 1.0 / (2.0 ** (1.0 / BPO) - 1.0)
    freqs = FMIN * 2.0 ** (np.arange(N_BINS) / BPO)
    lengths = np.ceil(Q * SR / freqs).astype(int)
    n_fft = int(2 ** np.ceil(np.log2(lengths.max())))
    K = np.zeros((N_BINS, n_fft), dtype=np.complex128)
    for k in range(N_BINS):
        L = int(lengths[k])
        t = np.arange(L) - (L - 1) / 2.0
        kern = np.hanning(L) * np.exp(2j * np.pi * freqs[k] * t / SR)
        kern /= np.abs(kern).sum()
        kern /= np.sqrt(L)
        s = (n_fft - L) // 2
        K[k, s:s + L] = kern
    return K.real.astype(np.float32), K.imag.astype(np.float32), n_fft


def _chunk_range(Kr, Ki, bins):
    """Contiguous range of 128-row K-chunks with any nonzero for these bins."""
    nz = (np.abs(Kr[bins]).max(axis=0) + np.abs(Ki[bins]).max(axis=0)) > 0
    idx = np.nonzero(nz)[0]
    return int(idx[0]) // P, int(idx[-1]) // P + 1


Kr, Ki, N_FFT = _build_cqt_kernels()
PAD = N_FFT // 2
FW = (N_SAMP + 2 * PAD) // P      # 628 free-dim width of column-major xp
assert (N_SAMP + 2 * PAD) % P == 0 and HOP == 4 * P

_A0, _A1 = _chunk_range(Kr, Ki, range(0, SPLIT_BIN))
_B0, _B1 = _chunk_range(Kr, Ki, range(SPLIT_BIN, N_BINS))
# Chunks are processed grouped by phase r = c % 4 (ascending), matching the
# order the four phase-DMAs of x land in SBUF, so the first matmuls only
# wait for the first phase slice instead of the whole signal. The four
# B-group chunks are interleaved into the same phase order so they never
# stall the PE FIFO waiting for a late phase.
CHUNKS_A = sorted(range(_A0, _A1), key=lambda c: (c % 4, c))  # 90 chunks
CHUNKS_B = sorted(range(_B0, _B1), key=lambda c: (c % 4, c))  # 4 chunks
NA = len(CHUNKS_A)
NB = len(CHUNKS_B)
SLAB = 15                         # WA chunks per weight DMA slab
N_SLABS = (NA + SLAB - 1) // SLAB


def _pack_weights():
    KrT = Kr.T  # [N_FFT, 84]
    KiT = Ki.T
    wa = np.zeros((P, NA * P), np.float32)
    for j, c in enumerate(CHUNKS_A):
        wa[:, j * P: j * P + SPLIT_BIN] = KrT[c * P:(c + 1) * P, :SPLIT_BIN]
        wa[:, j * P + SPLIT_BIN:(j + 1) * P] = KiT[c * P:(c + 1) * P, :SPLIT_BIN]
    wb = np.zeros((P, NB * MB), np.float32)
    for j, c in enumerate(CHUNKS_B):
        wb[:, j * MB: j * MB + NB_BINS] = KrT[c * P:(c + 1) * P, SPLIT_BIN:]
        wb[:, j * MB + 32: j * MB + 32 + NB_BINS] = KiT[c * P:(c + 1) * P, SPLIT_BIN:]
    return (wa * W_SCALE).astype(MM_NP), (wb * W_SCALE).astype(MM_NP)


WA, WB = _pack_weights()


def build_program():
    """One-core SPMD program (identical on all cores; data differs)."""
    nc = bacc.Bacc("TRN2", target_bir_lowering=False, debug=False,
                   enable_asserts=True)
    bf16 = MM_DT
    f32 = mybir.dt.float32

    x_in = nc.dram_tensor("x_in", [4, P, NI * FW // 4], bf16,
                          kind="ExternalInput").ap()
    wa_in = nc.dram_tensor("wa_in", [P, NA * P], bf16, kind="ExternalInput").ap()
    wb_in = nc.dram_tensor("wb_in", [P, NB * MB], bf16, kind="ExternalInput").ap()
    # [bins, items, T] so the result leaves SBUF as ONE contiguous DMA;
    # the host transposes to [items, bins, T] afterwards (free in numpy).
    out = nc.dram_tensor("out", [N_BINS, NI, T], f32, kind="ExternalOutput").ap()

    with tile.TileContext(nc) as tc, ExitStack() as ctx:
        sb = ctx.enter_context(tc.tile_pool(name="sb", bufs=1))
        ps = ctx.enter_context(tc.tile_pool(name="ps", bufs=1, space="PSUM"))

        # PE warmup: ~4us of dummy matmuls during the DMA head so HAM
        # unthrottles (1.2 -> 2.4 GHz) before the first real matmul.
        junk = sb.tile([P, 512], bf16, name="junk")
        nc.gpsimd.memset(junk[:], 0.0)
        psW = ps.tile([P, 504], f32, name="psW")
        for _ in range(6):
            nc.tensor.matmul(psW[:], lhsT=junk[:, :P], rhs=junk[:, :504],
                             start=True, stop=True)

        # input signal, phase-major: xt[p, r*628 + i*157 + q] = xp_i[(4q+r)*128+p]
        # Four phase DMAs issued from three different rings so the ~0.8us
        # issue slots don't serialize on one queue.
        QW = FW // 4  # 157
        xt = sb.tile([P, NI * FW], bf16, name="xt")
        issuers = [nc.sync, nc.scalar, nc.gpsimd, nc.gpsimd]
        for r in range(4):
            issuers[r].dma_start(xt[:, r * NI * QW:(r + 1) * NI * QW], x_in[r])
        xv = xt[:].rearrange("p (r i q) -> p r i q", r=4, i=NI)

        # group-B weights + slabbed group-A weights
        wbt = sb.tile([P, NB * MB], bf16, name="wbt")
        nc.scalar.dma_start(wbt[:], wb_in)
        slabs = []
        for s in range(N_SLABS):
            c0, c1 = s * SLAB, min((s + 1) * SLAB, NA)
            t = sb.tile([P, (c1 - c0) * P], bf16, name=f"wa{s}")
            nc.sync.dma_start(t[:], wa_in[:, c0 * P:c1 * P])
            slabs.append(t)

        # Load the Ln table set early: a [1,1] dummy Ln is the first ACT op,
        # so its ACT_TABLE_LOAD hoists into the DMA head; Copy (table filler
        # present in every set) then reuses the resident set.
        lnwarm = sb.tile([1, 1], f32, name="lnwarm")
        nc.scalar.activation(lnwarm[:], nc.const_aps.tensor(1.0, (1, 1)),
                             mybir.ActivationFunctionType.Ln)

        def rhs_for(c):
            # x is stored phase-deinterleaved (see pack_x): chunk c's moving
            # operand is contiguous in t, so the PE streams at full rate
            # (a stride-4 AP measured 3 cycles/column instead of 1).
            r, q0 = c % 4, c // 4
            return xv[:, r, :, q0: q0 + T]  # [128, NI, T]

        # group B first: 4 matmuls, output [64, NI, T]
        psB = ps.tile([MB, NI, T], f32, name="psB")
        for j, c in enumerate(CHUNKS_B):
            nc.tensor.matmul(psB[:], lhsT=wbt[:, j * MB:(j + 1) * MB],
                             rhs=rhs_for(c), start=(j == 0), stop=(j == NB - 1))

        # group A: 90 matmuls, output [128, NI, T] (re rows 0:64, im rows 64:128)
        psA = ps.tile([P, NI, T], f32, name="psA")
        for j, c in enumerate(CHUNKS_A):
            s, o = divmod(j, SLAB)
            nc.tensor.matmul(psA[:], lhsT=slabs[s][:, o * P:(o + 1) * P],
                             rhs=rhs_for(c), start=(j == 0), stop=(j == NA - 1))

        psAf = psA[:].rearrange("p i f -> p (i f)")
        psBf = psB[:].rearrange("p i f -> p (i f)")
        NT = NI * T

        # m2 = re^2 + im^2 for all 84 bins. ACT copies PSUM->SBUF (walrus
        # allows only one PSUM operand per DVE op); DVE squares and adds.
        # Copy is table-set-free filler, so the kernel's single table load
        # (Ln's set) hoists off the critical tail. (DVE tensor_tensor needs
        # equal base partitions for its SBUF inputs -> cre/cim mirror m2.)
        Copy = mybir.ActivationFunctionType.Copy
        cre = sb.tile([N_BINS, NT], f32, name="cre")
        cim = sb.tile([N_BINS, NT], f32, name="cim")
        m2 = sb.tile([N_BINS, NT], f32, name="m2")
        tmp = sb.tile([N_BINS, NT], f32, name="tmp")
        nc.scalar.activation(cre[SPLIT_BIN:], psBf[:NB_BINS], Copy)
        nc.scalar.activation(cim[SPLIT_BIN:], psBf[32:32 + NB_BINS], Copy)
        nc.vector.tensor_mul(m2[SPLIT_BIN:], cre[SPLIT_BIN:], cre[SPLIT_BIN:])
        nc.vector.tensor_mul(tmp[SPLIT_BIN:], cim[SPLIT_BIN:], cim[SPLIT_BIN:])
        nc.vector.tensor_add(m2[SPLIT_BIN:], m2[SPLIT_BIN:], tmp[SPLIT_BIN:])
        nc.vector.tensor_copy(cim[:SPLIT_BIN], psAf[SPLIT_BIN:])
        nc.scalar.activation(cre[:SPLIT_BIN], psAf[:SPLIT_BIN], Copy)
        nc.vector.tensor_mul(m2[:SPLIT_BIN], psAf[:SPLIT_BIN], cre[:SPLIT_BIN])
        nc.vector.tensor_mul(tmp[:SPLIT_BIN], psAf[SPLIT_BIN:], cim[:SPLIT_BIN])
        nc.vector.tensor_add(m2[:SPLIT_BIN], m2[:SPLIT_BIN], tmp[:SPLIT_BIN])

        # clamp at amin^2 (amplitude_to_db's amin on mag == amin^2 on mag^2)
        nc.vector.tensor_scalar_max(m2[:], m2[:], float(AMIN * W_SCALE) ** 2)

        # ln(m2c) for every element
        lnm = sb.tile([N_BINS, NT], f32, name="lnm")
        nc.scalar.activation(lnm[:], m2[:], mybir.ActivationFunctionType.Ln)

        # per-item ref^2: free-dim max within each item block, then
        # partition all-reduce so every partition holds the item max
        r1 = sb.tile([N_BINS, NI], f32, name="r1")
        nc.vector.tensor_reduce(r1[:], m2[:].rearrange("p (i f) -> p i f", i=NI),
                                axis=mybir.AxisListType.X, op=mybir.AluOpType.max)
        rall = sb.tile([N_BINS, NI], f32, name="rall")
        nc.gpsimd.partition_all_reduce(rall[:], r1[:], channels=N_BINS,
                                       reduce_op=bass_isa.ReduceOp.max)
        lnr = sb.tile([N_BINS, NI], f32, name="lnr")
        nc.scalar.activation(lnr[:], rall[:], mybir.ActivationFunctionType.Ln)

        # db = (ln(m2c) - ln(ref2c)) * 10/ln(10), clamped at -top_db
        db = sb.tile([N_BINS, NT], f32, name="db")
        for i in range(NI):
            nc.vector.tensor_scalar(db[:, i * T:(i + 1) * T],
                                    lnm[:, i * T:(i + 1) * T],
                                    lnr[:, i:i + 1], float(DB_SCALE),
                                    mybir.AluOpType.subtract,
                                    mybir.AluOpType.mult)

        nc.sync.dma_start(out.rearrange("k i t -> k (i t)"), db[:])

    nc.compile()
    return nc


def pack_x(x):
    """x [B, 64000] f32 -> per-core MM_DTYPE column-major packs [P, NI*FW]."""
    xp = np.pad(np.asarray(x, dtype=np.float32), ((0, 0), (PAD, PAD)))
    # phase-deinterleaved column-major: x_cm[b, p, r, q] = xp[b, (4q+r)*128+p]
    # so chunk c (= 4*q0 + r) streams contiguously in t (HOP = 4*128).
    x_cm = xp.reshape(B, FW // 4, 4, P).transpose(0, 3, 2, 1)  # [B,128,4,157]
    x_cm = x_cm.astype(MM_NP)
    packs = []
    for core in range(N_CORES):
        blk = x_cm[core * NI:(core + 1) * NI]           # [NI, 128, 4, 157]
        packs.append(np.ascontiguousarray(
            blk.transpose(2, 1, 0, 3).reshape(4, P, NI * (FW // 4))))
    return packs


_PROGRAM = None


def _get_program():
    global _PROGRAM
    if _PROGRAM is None:
        _PROGRAM = build_program()
    return _PROGRAM


def run(x, **spmd_kwargs):
    """Run on 8 NeuronCores; returns (output [32, 84, 126] f32, BassKernelResults)."""
    nc = _get_program()
    packs = pack_x(x)
    in_maps = [{"x_in": packs[i], "wa_in": WA, "wb_in": WB}
               for i in range(N_CORES)]
    res = run_bass_kernel_spmd(nc, in_maps, core_ids=list(range(N_CORES)),
                               **spmd_kwargs)
    out = np.concatenate([res.results[i]["out"].transpose(1, 0, 2)
                          for i in range(N_CORES)], axis=0)
    return np.ascontiguousarray(out.astype(np.float32)), res


def kernel(x):
    return run(x)[0]
